# revision 38
# baseline (speedup 1.0000x reference)
"""DiffTransformer layer on 8 TRN2 NeuronCores (v2).

Sharding: core c = (batch b=c//2, head-group g=c%2). Each core computes
q/k/v projections + differential attention for its 8 heads of its batch
(transposed [feature, seq] layout) in bf16, normalizes (subln), then a
pair AllToAll swaps normed attention halves so each core holds ALL 1024
attention channels for its two seq quarters ([g*256,+256) and
[512+g*256,+256)). Out-projection and FFN then run fully local; no
reduce collective.

Numerics: bf16 operands everywhere with fp32 PSUM accumulation. Softmax
without max-subtraction; scores for the 4 head-branches of a pt-tile
land in one 4-bank PSUM tile so ONE batched Exp ACTIVATE covers them.
Banded k-tiles write shifted columns (no unwritten-PSUM garbage);
causal diag masked multiplicatively on e. Softmax division deferred:
denominators ride the PV matmul as 64 ones-columns of va (broadcast for
free), folded into the subln RMS via scale invariance. rsqrt = ln+exp
(same ACT table set as Exp -> no table reloads). rstd broadcast via a
tiny matmul.
"""
import os
import sys
import numpy as np

for _p in ("/opt/trn_rl_repo", "/root/.axon_site/_ro/trn_rl_repo"):
    if os.path.isdir(_p) and _p not in sys.path:
        sys.path.append(_p)

B, S, D, H, HD, FF = 4, 1024, 1024, 16, 32, 4096
NCORES = 8
LAMBDA_INIT = 0.8 - 0.6 * float(np.exp(-0.3 * 12))
EPS = 1e-5
SCALE = float(HD) ** -0.5

SWAP16 = [((i + 16) % 32) for i in range(32)]

LAST_RESULT = None  # BassKernelResults of the most recent run (for test.py)
_PROGRAM = {}


def _kts(qc):
    # (k-tile index, diag-band offset or None=full) for a 512-wide q chunk
    if qc == 0:
        return [(0, 0), (1, 128), (2, 256), (3, 384)]
    return [(0, None), (1, None), (2, None), (3, None),
            (4, 0), (5, 128), (6, 256), (7, 384)]


def _pin_act_table(bacc_mod):
    """Restrict the ACT table-set chooser to natural_log_exp_and_others.

    The default pass picks the first set covering each activation, which
    ping-pongs between exp_and_others and natural_log (~1.3us reload per
    switch, 16+ switches). Every function this kernel uses (exp, ln,
    relu, copy, identity) is in natural_log_exp_and_others, so emptying
    the other sets (indices preserved -- walrus resolves set ids against
    the real act_info.json) forces a single load.
    """
    if getattr(bacc_mod, "_act_tables_pinned", False):
        return
    real = bacc_mod.get_activation_tables

    def only_ln_exp(arch):
        tabs = real(arch)
        return {name: (fns if name == "natural_log_exp_and_others"
                       else set())
                for name, fns in tabs.items()}

    bacc_mod.get_activation_tables = only_ln_exp
    bacc_mod._act_tables_pinned = True


def _build_program():
    import concourse.bacc as bacc
    import concourse.mybir as mybir
    from concourse import tile

    _pin_act_table(bacc)

    dt = mybir.dt
    f32, f32r = dt.float32, dt.float32r
    bf16 = dt.bfloat16
    Alu = mybir.AluOpType
    Act = mybir.ActivationFunctionType

    nc = bacc.Bacc("TRN2", target_bir_lowering=False, debug=False,
                   num_devices=NCORES)

    P = 128
    xT = nc.declare_dram_parameter("xT", [D, S], bf16, isOutput=False)
    wqT = nc.declare_dram_parameter("wqT", [D, 512], bf16, isOutput=False)
    wkT = nc.declare_dram_parameter("wkT", [D, 512], bf16, isOutput=False)
    wvT = nc.declare_dram_parameter("wvT", [D, 512], bf16, isOutput=False)
    woTf = nc.declare_dram_parameter("woTf", [D, D], bf16, isOutput=False)
    w1s = nc.declare_dram_parameter("w1s", [32, P, 1024], bf16, isOutput=False)
    w2T = nc.declare_dram_parameter("w2T", [FF, D], bf16, isOutput=False)
    b1c = nc.declare_dram_parameter("b1c", [P, 32], f32, isOutput=False)
    b2c = nc.declare_dram_parameter("b2c", [P, 8], f32, isOutput=False)
    rmswc = nc.declare_dram_parameter("rmswc", [P, 8], f32, isOutput=False)
    lam128 = nc.declare_dram_parameter("lam128", [P, 1], f32, isOutput=False)
    cosT = nc.declare_dram_parameter("cosT", [P, S], bf16, isOutput=False)
    sinS = nc.declare_dram_parameter("sinS", [P, S], bf16, isOutput=False)
    hsel2 = nc.declare_dram_parameter("hsel2", [P, P], bf16, isOutput=False)
    trimask = nc.declare_dram_parameter("trimask", [P, P], bf16, isOutput=False)
    qsel = nc.declare_dram_parameter("qsel", [P, 2], f32, isOutput=False)
    outT = nc.declare_dram_parameter("outT", [D, 512], f32, isOutput=True)

    RG = [[0, 1], [2, 3], [4, 5], [6, 7]]

    from contextlib import ExitStack

    with tile.TileContext(nc) as tc:
        with (
            tc.tile_pool(name="consts", bufs=1) as consts,
            tc.tile_pool(name="dram", bufs=1, space="DRAM") as dram,
            tc.tile_pool(name="aT", bufs=1) as atp,
            tc.tile_pool(name="w2r", bufs=1) as w2r_pool,
        ):
            _astk = ExitStack()
            wop = _astk.enter_context(tc.tile_pool(name="wo_sbp", bufs=1))
            qkpool = _astk.enter_context(tc.tile_pool(name="qk", bufs=1))
            vapool = _astk.enter_context(tc.tile_pool(name="vaug", bufs=1))
            attnpool = _astk.enter_context(tc.tile_pool(name="attn", bufs=1))
            # ---- constants -------------------------------------------
            hs_sb = consts.tile([P, P], bf16, tag="hs")
            tri_sb = consts.tile([P, P], bf16, tag="tri")
            lam_sb = consts.tile([P, 1], f32, tag="lam")
            b1_sb = consts.tile([P, 32], f32, tag="b1")
            b2_sb = consts.tile([P, 8], f32, tag="b2")
            rw_sb = consts.tile([P, 8], f32, tag="rw")
            qsel_sb = consts.tile([P, 2], f32, tag="qsel")
            nc.sync.dma_start(qsel_sb[:], qsel[:])
            nc.sync.dma_start(hs_sb[:], hsel2[:])
            nc.sync.dma_start(tri_sb[:], trimask[:])
            nc.sync.dma_start(lam_sb[:], lam128[:])
            nc.sync.dma_start(b1_sb[:], b1c[:])
            nc.sync.dma_start(b2_sb[:], b2c[:])
            nc.sync.dma_start(rw_sb[:], rmswc[:])
            eps_sb = consts.tile([1, 1], f32, tag="eps")
            nc.vector.memset(eps_sb[:], EPS)
            ones32 = consts.tile([P, 1], bf16, tag="o32")
            nc.vector.memset(ones32[:], 1.0 / 32.0)
            ones1f = consts.tile([1, P], f32, tag="o1f")
            nc.vector.memset(ones1f[:], 1.0)
            ones1r = consts.tile([1, P], f32r, tag="o1r")
            nc.vector.tensor_copy(ones1r[:], ones1f[:])

            # persistent attention-phase tiles
            qT = [qkpool.tile([P, S], bf16, tag=f"qT{i}", name=f"qT{i}")
                  for i in range(4)]
            kT = [qkpool.tile([P, S], bf16, tag=f"kT{i}", name=f"kT{i}")
                  for i in range(4)]
            va = [vapool.tile([P, 8 * P], bf16, tag=f"va{i}", name=f"va{i}")
                  for i in range(8)]
            attnT = [attnpool.tile([P, S], bf16, tag=f"at{i}", name=f"at{i}")
                     for i in range(4)]
            aTr = [atp.tile([P, 512], bf16, tag=f"ar{i}", name=f"ar{i}")
                   for i in range(8)]

            # pair-exchange staging: one AllGather per (qc, pt); each core
            # contributes its 128 channels for the full qc seq-half, the
            # seq-quarter selection happens at consume time via per-core
            # 0/1 blend constants (qsel).
            ag_in = [[dram.tile([P, 512], bf16, name=f"agin{qc}_{pt}")
                      for pt in range(4)] for qc in range(2)]
            ag_out = [[dram.tile([2, P, 512], bf16, name=f"agout{qc}_{pt}")
                       for pt in range(4)] for qc in range(2)]

            # wo weights: full [D, D] resident (2MB bf16). DMAs issued
            # later, throttled behind attention staging points, so they
            # don't steal HBM bandwidth from the startup x/wq loads.
            wo_sb = [wop.tile([P, D], bf16, tag=f"wo{i}", name=f"wo{i}")
                     for i in range(8)]

            # ---- phase 1+2: load xT / weights, project q,k then v ----
            with (
                tc.tile_pool(name="xw", bufs=1) as xw,
                tc.tile_pool(name="proj_ps", bufs=6, space="PSUM") as pps,
                tc.tile_pool(name="rtmp", bufs=6) as rtmp,
            ):
                xt = [xw.tile([P, S], bf16, tag=f"x{i}", name=f"x{i}")
                      for i in range(8)]
                cos_sb = xw.tile([P, S], bf16, tag="cos")
                sin_sb = xw.tile([P, S], bf16, tag="sin")
                wq_sb = [xw.tile([P, 512], bf16, tag=f"wq{i}", name=f"wqs{i}")
                         for i in range(8)]
                wk_sb = [xw.tile([P, 512], bf16, tag=f"wk{i}", name=f"wks{i}")
                         for i in range(8)]
                wv_sb = [xw.tile([P, 512], bf16, tag=f"wv{i}", name=f"wvs{i}")
                         for i in range(8)]
                # interleave x/wq so the first q matmuls can start after
                # ~2 chunks instead of after the full 3MB
                for i in range(8):
                    nc.sync.dma_start(xt[i][:], xT[i * P:(i + 1) * P, :])
                    nc.sync.dma_start(wq_sb[i][:], wqT[i * P:(i + 1) * P, :])
                nc.sync.dma_start(cos_sb[:], cosT[:])
                nc.sync.dma_start(sin_sb[:], sinS[:])
                for i in range(8):
                    nc.sync.dma_start(wk_sb[i][:], wkT[i * P:(i + 1) * P, :])
                for i in range(8):
                    nc.sync.dma_start(wv_sb[i][:], wvT[i * P:(i + 1) * P, :])

                def rope_evict(ps, dst, n0):
                    # dst = ps*cos + shuffle16(ps)*sin, all bf16 via staging
                    ev = rtmp.tile([P, 512], bf16, tag="ev", name="ev")
                    nc.scalar.copy(ev[:], ps[:])
                    nc.vector.tensor_tensor(
                        dst, ev[:], cos_sb[:, n0:n0 + 512], Alu.mult)
                    tmp = rtmp.tile([P, 512], bf16, tag="rt", name="rt")
                    nc.vector.stream_shuffle(tmp[:], ev[:], SWAP16)
                    nc.vector.tensor_tensor(
                        tmp[:], tmp[:], sin_sb[:, n0:n0 + 512], Alu.mult)
                    nc.vector.tensor_tensor(dst, dst, tmp[:], Alu.add)

                def project_one(wsb, dstT, mt):
                    # one LDWEIGHTS feeds two 512-seq matmuls
                    ps0 = pps.tile([P, 512], f32, tag="ps", name="ps")
                    ps1 = pps.tile([P, 512], f32, tag="ps", name="ps")
                    for kd in range(8):
                        lh = wsb[kd][:, mt * P:(mt + 1) * P]
                        nc.tensor.matmul(ps0[:], lhsT=lh,
                                         rhs=xt[kd][:, 0:512],
                                         start=(kd == 0), stop=(kd == 7))
                        nc.tensor.matmul(ps1[:], lhsT=lh,
                                         rhs=xt[kd][:, 512:1024],
                                         start=(kd == 0), stop=(kd == 7))
                    rope_evict(ps0, dstT[mt][:, 0:512], 0)
                    rope_evict(ps1, dstT[mt][:, 512:1024], 512)

                def project_v(st):
                    ps = pps.tile([P, 512], f32, tag="ps", name="ps")
                    for kd in range(8):
                        nc.tensor.matmul(
                            ps[:],
                            lhsT=xt[kd][:, st * P:(st + 1) * P],
                            rhs=wv_sb[kd][:],
                            start=(kd == 0), stop=(kd == 7))
                    nc.vector.memset(va[st][:], 1.0)
                    va3 = va[st][:].rearrange("p (h e) -> p h e", h=8, e=128)
                    nc.any.tensor_copy(
                        va3[:, :, 0:64],
                        ps[:].rearrange("p (h e) -> p h e", h=8, e=64))

                # q first (x/wq arrive first), then k, then v — keeps the
                # PE fed while the later weight DMAs stream in
                for mt in range(4):
                    project_one(wq_sb, qT, mt)
                for mt in range(4):
                    project_one(wk_sb, kT, mt)
                for st in range(8):
                    project_v(st)

            # w2 resident (8MB bf16): DMAs issued inside the attention
            # loop (throttled behind staging points).
            w2res = [w2r_pool.tile([P, 1024], bf16, tag=f"w2_{i}",
                                   name=f"w2_{i}") for i in range(32)]

            # ---- phase 3: differential attention + norm + exchange ----
            with (
                tc.tile_pool(name="st_ps", bufs=1, space="PSUM") as stp,
                tc.tile_pool(name="pv_ps", bufs=4, space="PSUM") as pvp,
                tc.tile_pool(name="epool", bufs=2) as epool,
                tc.tile_pool(name="npool", bufs=2) as npool,
                tc.tile_pool(name="small", bufs=4) as small,
            ):
                # deferred ACT tail of the previous group's norm, so the
                # next group's exps aren't blocked behind it in the ACT
                # FIFO (ln waits on a long DVE chain).
                pending_fin = []

                def emit_fin():
                    while pending_fin:
                        pending_fin.pop(0)()

                for qc in (1, 0):
                    q0 = qc * 512
                    kts = _kts(qc)
                    last_kt = kts[-1][0]
                    for pt in range(4):
                        pvs = [pvp.tile([P, 512], f32, tag="pv", name="pv")
                               for _ in range(4)]

                        def emit_pv(pend, pvs=pvs, pt=pt, last_kt=last_kt):
                            kt, off, e3 = pend
                            j0 = 0 if off is None else off
                            for gq in range(4):
                                h_loc = (pt * P + gq * 32) // 64
                                nc.tensor.matmul(
                                    pvs[gq][:, j0:512],
                                    lhsT=va[kt][:, h_loc * P:(h_loc + 1) * P],
                                    rhs=e3[:, gq, 0:512 - j0],
                                    start=(kt == 0), stop=(kt == last_kt))

                        # software-pipelined: PV(kt-1) is issued AFTER
                        # scores(kt) so the PE never sits behind exp(kt)
                        pend = None
                        for ik, (kt, off) in enumerate(kts):
                            j0 = 0 if off is None else off
                            W = 512 - j0
                            stP = stp.tile([P, 2048], f32, tag="st",
                                           name="stP")
                            st3 = stP[:].rearrange("p (g n) -> p g n", g=4,
                                                   n=512)
                            for gq in range(4):
                                nc.tensor.matmul(
                                    st3[:, gq, 0:W],
                                    lhsT=kT[pt][gq * 32:(gq + 1) * 32,
                                                kt * P:(kt + 1) * P],
                                    rhs=qT[pt][gq * 32:(gq + 1) * 32,
                                               q0 + j0:q0 + 512],
                                    start=True, stop=True,
                                    tile_position=(gq * 32, 0))
                            if pend is not None:
                                emit_pv(pend)
                            e = epool.tile([P, 2048], bf16, tag="e", name="e")
                            e3 = e[:].rearrange("p (g n) -> p g n", g=4,
                                                n=512)
                            nc.scalar.activation(
                                e3[:, :, 0:W], st3[:, :, 0:W], Act.Exp,
                                scale=SCALE)
                            if ik == 2:
                                # previous group's deferred norm tail: by
                                # now its sq is long done, so the msBC
                                # matmul doesn't head-of-line-block our
                                # scores in the PE FIFO
                                emit_fin()
                            if off is not None:
                                # multiplicative causal mask on the diag
                                # block (first 128 shifted cols); one op
                                # for all 4 gq via a stride-0 broadcast AP
                                nc.vector.tensor_tensor(
                                    e3[:, :, 0:P], e3[:, :, 0:P],
                                    tri_sb[:].unsqueeze(1).broadcast_to(
                                        [P, 4, P]),
                                    Alu.mult)
                            pend = (kt, off, e3)
                        emit_pv(pend)

                        # ---- subln norm (DVE part) for this (qc, pt) ----
                        # pvs[2h] rows 0:64 = a1(head h), 64:128 = z1 bcast
                        # pvs[2h+1]: a2 / z2 bcast
                        bcz1 = npool.tile([P, 512], f32r, tag="bz1")
                        bcz2 = npool.tile([P, 512], f32r, tag="bz2")
                        t1 = npool.tile([P, 512], f32r, tag="t1")
                        t2 = npool.tile([P, 512], f32r, tag="t2")
                        zzb = npool.tile([P, 512], f32, tag="zzb")
                        for hp in range(2):
                            r = slice(hp * 64, hp * 64 + 64)
                            nc.vector.tensor_copy(bcz1[r, :],
                                                  pvs[2 * hp][64:128, :])
                            nc.vector.tensor_copy(bcz2[r, :],
                                                  pvs[2 * hp + 1][64:128, :])
                            nc.vector.tensor_tensor(
                                t1[r, :], pvs[2 * hp][0:64, :], bcz2[r, :],
                                Alu.mult)
                            nc.vector.tensor_tensor(
                                t2[r, :], pvs[2 * hp + 1][0:64, :],
                                bcz1[r, :], Alu.mult)
                        nc.vector.tensor_tensor(zzb[:], bcz1[:], bcz2[:],
                                                Alu.mult)
                        negw = npool.tile([P, 512], f32r, tag="negw")
                        nc.vector.scalar_tensor_tensor(
                            negw[:], in0=t2[:], scalar=lam_sb[:, 0:1],
                            in1=t1[:], op0=Alu.mult, op1=Alu.subtract)
                        # sq = (negw*0.125)*negw; hselBC carries 0.125
                        # -> ms = sum(negw^2)/64 per head-half
                        sq = npool.tile([P, 512], bf16, tag="sq")
                        nc.vector.scalar_tensor_tensor(
                            sq[:], in0=negw[:], scalar=0.125, in1=negw[:],
                            op0=Alu.mult, op1=Alu.mult)
                        # zz side-path (DVE only, cheap) stays immediate
                        q1b = npool.tile([P, 512], f32, tag="q1b")
                        nc.vector.scalar_tensor_tensor(
                            q1b[:], in0=zzb[:], scalar=EPS, in1=zzb[:],
                            op0=Alu.mult, op1=Alu.mult)
                        # allocate msBC now (keeps pvp rotation aligned)
                        # but ISSUE its matmul + the ACT tail deferred
                        msBC = pvp.tile([P, 512], f32, tag="pv", name="msps")

                        def fin(qc=qc, pt=pt, q0=q0, negw=negw, sq=sq,
                                q1b=q1b, msBC=msBC):
                            # hselBC sums squares over each head-half AND
                            # broadcasts to all 64 rows of the half
                            nc.tensor.matmul(msBC[:], lhsT=hs_sb[:],
                                             rhs=sq[:], start=True,
                                             stop=True)
                            msb2 = npool.tile([P, 512], f32, tag="msb2")
                            nc.vector.tensor_tensor(msb2[:], q1b[:],
                                                    msBC[:], Alu.add)
                            # rstd = msb2^-0.5 via ln+exp (same ACT table)
                            lnv = npool.tile([P, 512], f32, tag="lnv")
                            nc.scalar.activation(lnv[:], msb2[:], Act.Ln)
                            rstd = npool.tile([P, 512], f32r, tag="rstd")
                            nc.scalar.activation(rstd[:], lnv[:], Act.Exp,
                                                 scale=-0.5)
                            nc.vector.scalar_tensor_tensor(
                                attnT[pt][:, q0:q0 + 512],
                                in0=negw[:], scalar=-(1.0 - LAMBDA_INIT),
                                in1=rstd[:], op0=Alu.mult, op1=Alu.mult)
                            # stage + exchange this pt's channels
                            nc.sync.dma_start(ag_in[qc][pt][:],
                                              attnT[pt][:, q0:q0 + 512])
                            nc.gpsimd.collective_compute(
                                "AllGather",
                                Alu.bypass,
                                replica_groups=RG,
                                ins=[ag_in[qc][pt].opt()],
                                outs=[ag_out[qc][pt].opt()],
                            )
                            # throttled weight loads: the sync queue is
                            # FIFO, so these only start once the staging
                            # DMA above resolves -- keeps startup HBM
                            # bandwidth for x/wq/wk/wv.
                            sp = qc * 4 + pt
                            if sp < 8:
                                nc.sync.dma_start(
                                    wo_sb[sp][:],
                                    woTf[sp * P:(sp + 1) * P, :])
                            for i in range(4 * sp, 4 * sp + 4):
                                nc.sync.dma_start(
                                    w2res[i][:],
                                    w2T[i * P:(i + 1) * P, :])

                        pending_fin.append(fin)
                emit_fin()


            # ---- phase 4: out-projection on local seq quarters ----
            with (
                tc.tile_pool(name="rcp", bufs=1) as rcp,
                tc.tile_pool(name="wo_ps", bufs=4, space="PSUM") as wops,
            ):
                rc = [[rcp.tile([P, 256], bf16, tag=f"rc{qc}_{kc}",
                                name=f"rc{qc}_{kc}") for kc in range(8)]
                      for qc in range(2)]
                rb = [[rcp.tile([P, 512], bf16, tag=f"rb{qc}_{kc}",
                                name=f"rb{qc}_{kc}") for kc in range(8)]
                      for qc in range(2)]
                for qc in (1, 0):
                    for kc in range(8):
                        # chunk kc = global channels [kc*128,...): rank
                        # slot kc//4, pt kc%4
                        nc.sync.dma_start(
                            rb[qc][kc][:], ag_out[qc][kc % 4][kc // 4])
                        # select my seq quarter: qsel col0/col1 are 1-g / g
                        tmpq = rcp.tile([P, 256], bf16, tag="tmpq",
                                        name="tmpq", bufs=4)
                        nc.vector.tensor_scalar_mul(
                            tmpq[:], rb[qc][kc][:, 0:256], qsel_sb[:, 0:1])
                        nc.vector.scalar_tensor_tensor(
                            rc[qc][kc][:], in0=rb[qc][kc][:, 256:512],
                            scalar=qsel_sb[:, 1:2], in1=tmpq[:],
                            op0=Alu.mult, op1=Alu.add)
                for qc in (1, 0):
                    for mo in range(8):
                        ps = wops.tile([P, 256], f32, tag="wops", name="wops")
                        for kc in range(8):
                            nc.tensor.matmul(
                                ps[:],
                                lhsT=wo_sb[kc][:, mo * P:(mo + 1) * P],
                                rhs=rc[qc][kc][:],
                                start=(kc == 0), stop=(kc == 7))
                        nc.scalar.copy(
                            aTr[mo][:, qc * 256:(qc + 1) * 256], ps[:])
            _astk.close()

            # ---- phase 5: FFN + residual + final RMS on seq quarters ----
            with (
                tc.tile_pool(name="h1", bufs=1) as h1p,
                tc.tile_pool(name="w1p", bufs=8) as w1p,
                tc.tile_pool(name="h1_ps", bufs=4, space="PSUM") as h1ps,
            ):
                h1 = [h1p.tile([P, 512], bf16, tag=f"h1_{i}", name=f"h1_{i}")
                      for i in range(32)]
                for mf in range(32):
                    wt = w1p.tile([P, 1024], bf16, tag="w1t", name="w1t")
                    nc.sync.dma_start(wt[:], w1s[mf, :, :])
                    ps = h1ps.tile([P, 512], f32, tag="h1ps", name="h1ps")
                    for kd in range(8):
                        nc.tensor.matmul(
                            ps[:], lhsT=wt[:, kd * P:(kd + 1) * P],
                            rhs=aTr[kd][:], start=(kd == 0), stop=(kd == 7))
                    nc.scalar.activation(h1[mf][:], ps[:], Act.Relu,
                                         bias=b1_sb[:, mf:mf + 1])

                # h2 mo-outer (w2 resident) so the final-RMS pipeline
                # overlaps the remaining h2 matmuls.
                with (
                    tc.tile_pool(name="h2_ps", bufs=2, space="PSUM") as h2ps,
                    tc.tile_pool(name="ms_ps", bufs=1, space="PSUM") as msp,
                    tc.tile_pool(name="yT", bufs=1) as ytp,
                    tc.tile_pool(name="fin", bufs=4) as finp,
                    tc.tile_pool(name="sm2", bufs=1) as sm2,
                ):
                    yt = [ytp.tile([P, 512], f32r, tag=f"y{i}", name=f"y{i}")
                          for i in range(8)]
                    msF = msp.tile([P, 512], f32, tag="msF", name="msF")
                    for mo in range(8):
                        ps2 = h2ps.tile([P, 512], f32, tag="h2ps",
                                        name="h2ps")
                        for kf in range(32):
                            nc.tensor.matmul(
                                ps2[:], lhsT=w2res[kf][:, mo * P:(mo + 1) * P],
                                rhs=h1[kf][:], start=(kf == 0),
                                stop=(kf == 31))
                        nc.vector.scalar_tensor_tensor(
                            yt[mo][:], in0=ps2[:],
                            scalar=b2_sb[:, mo:mo + 1], in1=aTr[mo][:],
                            op0=Alu.add, op1=Alu.add)
                        sqf = finp.tile([P, 512], bf16, tag="fsq",
                                        name="fsq")
                        nc.vector.scalar_tensor_tensor(
                            sqf[:], in0=yt[mo][:], scalar=1.0 / 32.0,
                            in1=yt[mo][:], op0=Alu.mult, op1=Alu.mult)
                        nc.tensor.matmul(msF[0:1, :], lhsT=ones32[:],
                                         rhs=sqf[:], start=(mo == 0),
                                         stop=(mo == 7))
                    # rstd = (mean+eps)^-0.5 via ln+exp; bcast via matmul
                    lnF = sm2.tile([1, 512], f32, tag="lnF")
                    nc.scalar.activation(lnF[:], msF[0:1, :], Act.Ln,
                                         bias=eps_sb[:])
                    frstd = sm2.tile([1, 512], f32r, tag="frstd")
                    nc.scalar.activation(frstd[:], lnF[:], Act.Exp,
                                         scale=-0.5)
                    fbcr = msp.tile([P, 512], f32, tag="fbcr", name="fbcr")
                    nc.tensor.matmul(fbcr[:], lhsT=ones1r[:], rhs=frstd[:],
                                     start=True, stop=True)
                    for mo in range(8):
                        ot = finp.tile([P, 512], f32, tag="fot", name="fot")
                        nc.vector.scalar_tensor_tensor(
                            ot[:], in0=yt[mo][:], scalar=rw_sb[:, mo:mo + 1],
                            in1=fbcr[:], op0=Alu.mult, op1=Alu.mult)
                        nc.sync.dma_start(outT[mo * P:(mo + 1) * P, :], ot[:])

    nc.compile()
    return nc


def _host_prep(inputs):
    import ml_dtypes
    bf = ml_dtypes.bfloat16
    x = np.asarray(inputs["x"], dtype=np.float32)
    Wq = np.asarray(inputs["Wq"], dtype=np.float32)
    Wk = np.asarray(inputs["Wk"], dtype=np.float32)
    Wv = np.asarray(inputs["Wv"], dtype=np.float32)
    Wo = np.asarray(inputs["Wo"], dtype=np.float32)
    W1 = np.asarray(inputs["W1"], dtype=np.float32)
    b1 = np.asarray(inputs["b1"], dtype=np.float32)
    W2 = np.asarray(inputs["W2"], dtype=np.float32)
    b2 = np.asarray(inputs["b2"], dtype=np.float32)
    rmsw = np.asarray(inputs["rms_weight"], dtype=np.float32)
    lam = float(np.exp(np.dot(np.asarray(inputs["lambda_q1"], np.float64),
                              np.asarray(inputs["lambda_k1"], np.float64)))
                - np.exp(np.dot(np.asarray(inputs["lambda_q2"], np.float64),
                                np.asarray(inputs["lambda_k2"], np.float64)))
                + LAMBDA_INIT)

    half = HD // 2
    freqs = (1.0 / (10000.0 ** (np.arange(half, dtype=np.float32)
                                / np.float32(half)))).astype(np.float32)
    ang = (np.arange(S, dtype=np.float32)[:, None] * freqs[None, :])
    cos16 = np.cos(ang.astype(np.float32)).T.astype(np.float32)
    sin16 = np.sin(ang.astype(np.float32)).T.astype(np.float32)

    cosT = np.ascontiguousarray(
        np.tile(np.concatenate([cos16, cos16], 0), (4, 1))).astype(bf)
    sinS = np.ascontiguousarray(
        np.tile(np.concatenate([-sin16, sin16], 0), (4, 1))).astype(bf)
    perm32 = np.concatenate([np.arange(0, 32, 2), np.arange(1, 32, 2)])

    hsel2 = np.zeros((128, 128), np.float32)
    hsel2[0:64, 0:64] = 0.125
    hsel2[64:128, 64:128] = 0.125
    trimask = (np.arange(128)[:, None] <= np.arange(128)[None, :]).astype(
        np.float32)

    b1c = np.ascontiguousarray(b1.reshape(32, 128).T)
    b2c = np.ascontiguousarray(b2.reshape(8, 128).T)
    rmswc = np.ascontiguousarray(rmsw.reshape(8, 128).T)
    lam128 = np.full((128, 1), lam, np.float32)
    # w1s[mf][p, kd*128+j] = W1.T[kd*128+p, mf*128+j]
    w1s = np.ascontiguousarray(
        W1.T.reshape(8, 128, 32, 128).transpose(2, 1, 0, 3)
        .reshape(32, 128, 1024).astype(bf))
    w2T = np.ascontiguousarray(W2.T.astype(bf))
    woTf = np.ascontiguousarray(Wo.T.astype(bf))

    in_maps = []
    for c in range(NCORES):
        b, g = c // 2, c % 2
        chans = np.arange(g * 512, (g + 1) * 512)
        permed = np.concatenate(
            [c0 * 32 + perm32 for c0 in range(g * 16, (g + 1) * 16)])
        qsel_np = np.zeros((128, 2), np.float32)
        qsel_np[:, 0] = 1.0 - g
        qsel_np[:, 1] = g
        in_maps.append({
            "qsel": qsel_np,
            "xT": np.ascontiguousarray(x[b].T.astype(bf)),
            "wqT": np.ascontiguousarray(Wq[permed, :].T.astype(bf)),
            "wkT": np.ascontiguousarray(Wk[permed, :].T.astype(bf)),
            "wvT": np.ascontiguousarray(Wv[chans, :].T.astype(bf)),
            "woTf": woTf,
            "w1s": w1s, "w2T": w2T,
            "b1c": b1c, "b2c": b2c, "rmswc": rmswc, "lam128": lam128,
            "cosT": cosT, "sinS": sinS,
            "hsel2": hsel2.astype(bf),
            "trimask": trimask.astype(bf),
        })
    return in_maps


def kernel(**inputs):
    global LAST_RESULT
    from concourse.bass_utils import run_bass_kernel_spmd

    if "nc" not in _PROGRAM:
        _PROGRAM["nc"] = _build_program()
    nc = _PROGRAM["nc"]

    in_maps = _host_prep(inputs)
    trace = bool(int(os.environ.get("KERNEL_TRACE", "0")))
    res = run_bass_kernel_spmd(nc, in_maps, list(range(NCORES)), trace=trace)
    LAST_RESULT = res

    out = np.empty((B, S, D), np.float32)
    for c in range(NCORES):
        b, g = c // 2, c % 2
        o = res.results[c]["outT"]
        out[b, g * 256:(g + 1) * 256, :] = o[:, 0:256].T
        out[b, 512 + g * 256:512 + (g + 1) * 256, :] = o[:, 256:512].T
    return out
